# revision 24
# baseline (speedup 1.0000x reference)
"""TRN2 Bass/Tile kernel for nn_NoFoDifformer (8-core SPMD, row-sharded).

Strategy (per core m, rows R_m = [m*1024, (m+1)*1024)):
  - feat-encoder MLP + LN/attention/FFN computed row-sharded in fp32.
  - pass 1: utx^T partial = sum_nt h[nt]^T @ u_panel  (fp32r matmuls, N=256)
    while streaming u panels; each panel is also cast-written (SWDGE
    fp32->bf16) to a DRAM scratch u16 for pass 2.
  - new_e (sine eigen-encoding) is jt-sharded across cores: each core
    computes 1/8 of new_e, then a tiny AllGather assembles it.
  - one fp32 AllReduce of [utx^T | kTv] (4.06 MB).
  - pass 2: h_fur^T = sum_jt g16[jt]^T @ u16T[jt] in bf16, with u16T tiles
    loaded via HWDGE DMA-transpose from the bf16 scratch.
  - attention chain (q/k/v/kTv/att) in true fp32 (it dominates the output
    scale ~3e4); bf16 error on the u path is ~1e-6 of output scale.
"""

import numpy as np

import concourse.bacc as bacc
import concourse.mybir as mybir
import concourse.tile as tile
from concourse.bass_utils import run_bass_kernel_spmd
from concourse.masks import make_identity

F32 = mybir.dt.float32
F32R = mybir.dt.float32r
BF16 = mybir.dt.bfloat16
AF = mybir.ActivationFunctionType
ALU = mybir.AluOpType

NCORES = 8
N = 8192
NFEAT = 512
HID = 256
C = 128
DIM = 32
KPOW = 10
ROWS = N // NCORES      # 1024 rows per core
NT = ROWS // 128        # 8 row tiles
JT = N // 128           # 64 column tiles
PW = 512                # pass-1 panel width
JP = N // PW            # 16 panels
JTC = JT // NCORES      # 8 jt per core for new_e sharding
LN_EPS = 1e-5

TWO_PI = 6.283185307179586
INV_2PI = 1.0 / TWO_PI
CW_C1 = 6.28125
CW_C2 = float(np.float32(TWO_PI - CW_C1))
CW_C3 = TWO_PI - CW_C1 - CW_C2
MAGIC = 12582912.0      # 1.5 * 2**23, round-to-nearest trick
HALF_PI = float(np.float32(np.pi / 2))
PI_F = float(np.float32(np.pi))

WEIGHT_NAMES = [
    ("fe_w1", [NFEAT, HID]), ("fe_b1", [HID]),
    ("fe_w2", [HID, C]), ("fe_b2", [C]),
    ("eig_w", [KPOW, DIM + 1]), ("eig_b", [KPOW]), ("alpha_w", [KPOW]),
    ("mha_g", [C]), ("mha_b", [C]), ("ffn_g", [C]), ("ffn_b", [C]),
    ("wq", [C, C]), ("bq", [C]), ("wk", [C, C]), ("bk", [C]),
    ("wv", [C, C]), ("bv", [C]), ("wo", [C, C]), ("bo", [C]),
    ("f1_w", [C, C]), ("f1_b", [C]), ("f2_w", [C, C]), ("f2_b", [C]),
]


def _build(nc):
    io = {}
    io["u_s"] = nc.dram_tensor("u_s", [ROWS, N], F32, kind="ExternalInput")
    io["x_s"] = nc.dram_tensor("x_s", [ROWS, NFEAT], F32, kind="ExternalInput")
    io["e_js"] = nc.dram_tensor("e_js", [128, JTC], F32, kind="ExternalInput")
    for name, shape in WEIGHT_NAMES:
        io[name] = nc.dram_tensor(name, shape, F32, kind="ExternalInput")
    y = nc.dram_tensor("y", [ROWS, C], F32, kind="ExternalOutput")

    div_const = nc.inline_tensor(
        np.tile(np.arange(1, DIM // 2 + 1, dtype=np.float32), (128, 1)), name="divc"
    )

    with tile.TileContext(nc) as tc:
        with (
            tc.tile_pool(name="persist", bufs=1) as per,
            tc.tile_pool(name="pan", bufs=3) as pan,
            tc.tile_pool(name="xchunk", bufs=2) as xchunk,
            tc.tile_pool(name="u16t", bufs=6) as u16t_pool,
            tc.tile_pool(name="g16p", bufs=4) as g16_pool,
            tc.tile_pool(name="stats", bufs=4) as stats,
            tc.tile_pool(name="psum_sm", bufs=3, space="PSUM") as psum_sm,
            tc.tile_pool(name="psum_tr", bufs=2, space="PSUM") as psum_tr,
            tc.tile_pool(name="psum_acc", bufs=1, space="PSUM") as psum_acc,
            tc.tile_pool(name="dram", bufs=1, space="DRAM") as dram,
        ):
            rg = [list(range(NCORES))]

            # ---------------- constants / weights to SBUF ----------------
            ident = per.tile([128, 128], F32, tag="ident")
            make_identity(nc, ident[:])
            ident16 = per.tile([128, 128], BF16, tag="ident16")
            make_identity(nc, ident16[:])

            eps_sb = per.tile([128, 1], F32, tag="eps_sb")
            nc.vector.memset(eps_sb[:], LN_EPS)

            div_sb = per.tile([128, DIM // 2], F32, tag="div_sb")
            nc.scalar.dma_start(out=div_sb[:], in_=div_const.ap())

            def bcast(name, width, tag):
                t = per.tile([128, width], F32, tag=tag)
                src = io[name].ap()
                if len(src.shape) > 1:
                    src = src.rearrange("a b -> (a b)")
                nc.scalar.dma_start(out=t[:], in_=src.partition_broadcast(128))
                return t

            def per_part(name, tag):
                t = per.tile([128, 1], F32, tag=tag)
                nc.scalar.dma_start(out=t[:], in_=io[name].ap().rearrange("(p o) -> p o", o=1))
                return t

            # encoder weights: w1 as [128(fpart), 4(ft), HID]; w2 as [128(hpart), 2(ht), C]
            w1_sb = per.tile([128, NFEAT // 128, HID], F32, tag="w1_sb")
            nc.sync.dma_start(out=w1_sb[:], in_=io["fe_w1"].ap().rearrange("(t p) h -> p t h", p=128))
            w2_sb = per.tile([128, HID // 128, C], F32, tag="w2_sb")
            nc.sync.dma_start(out=w2_sb[:], in_=io["fe_w2"].ap().rearrange("(t p) c -> p t c", p=128))
            b1_sb = per.tile([128, HID // 128], F32, tag="b1_sb")
            nc.sync.dma_start(out=b1_sb[:], in_=io["fe_b1"].ap().rearrange("(t p) -> p t", p=128))
            b2_bc = bcast("fe_b2", C, "b2_bc")

            wq_sb = per.tile([128, C], F32, tag="wq_sb")
            nc.sync.dma_start(out=wq_sb[:], in_=io["wq"].ap())
            wk_sb = per.tile([128, C], F32, tag="wk_sb")
            nc.sync.dma_start(out=wk_sb[:], in_=io["wk"].ap())
            wv_sb = per.tile([128, C], F32, tag="wv_sb")
            nc.sync.dma_start(out=wv_sb[:], in_=io["wv"].ap())
            wo_sb = per.tile([128, C], F32, tag="wo_sb")
            nc.sync.dma_start(out=wo_sb[:], in_=io["wo"].ap())
            f1w_sb = per.tile([128, C], F32, tag="f1w_sb")
            nc.sync.dma_start(out=f1w_sb[:], in_=io["f1_w"].ap())
            f2w_sb = per.tile([128, C], F32, tag="f2w_sb")
            nc.sync.dma_start(out=f2w_sb[:], in_=io["f2_w"].ap())

            bq_pp = per_part("bq", "bq_pp")
            bo_pp = per_part("bo", "bo_pp")
            f1b_pp = per_part("f1_b", "f1b_pp")
            f2b_pp = per_part("f2_b", "f2b_pp")
            bk_bc = bcast("bk", C, "bk_bc")
            bv_bc = bcast("bv", C, "bv_bc")
            mhag_bc = bcast("mha_g", C, "mhag_bc")
            mhab_bc = bcast("mha_b", C, "mhab_bc")
            ffng_bc = bcast("ffn_g", C, "ffng_bc")
            ffnb_bc = bcast("ffn_b", C, "ffnb_bc")

            # ---------------- feat encoder: h = relu(x@w1+b1)@w2+b2 ----------------
            # x^T tiles [f_part, 4(ft), n]
            xT = per.tile([128, NFEAT // 128, ROWS], F32, tag="xT")
            x_r = io["x_s"].ap().rearrange("(t p) f -> p t f", p=128)
            for nt in range(NT):
                xc = xchunk.tile([128, NFEAT], F32, tag="xc")
                nc.sync.dma_start(out=xc[:], in_=x_r[:, nt, :])
                for ft in range(NFEAT // 128):
                    tp = psum_tr.tile([128, 128], F32, tag="tr")
                    nc.tensor.transpose(tp[:], xc[:, ft * 128:(ft + 1) * 128], ident[:])
                    nc.vector.tensor_copy(out=xT[:, ft, nt * 128:(nt + 1) * 128], in_=tp[:])

            # t1^T [hid_part, 2(ht), n] = relu(w1^T x^T + b1)
            t1T = per.tile([128, HID // 128, ROWS], F32, tag="t1T")
            for ht in range(HID // 128):
                for nch in range(ROWS // 512):
                    ps = psum_sm.tile([128, 512], F32, tag="ps_sm")
                    for ft in range(NFEAT // 128):
                        nc.tensor.matmul(
                            ps[:], lhsT=w1_sb[:, ft, ht * 128:(ht + 1) * 128],
                            rhs=xT[:, ft, nch * 512:(nch + 1) * 512],
                            start=(ft == 0), stop=(ft == NFEAT // 128 - 1),
                        )
                    nc.scalar.activation(
                        out=t1T[:, ht, nch * 512:(nch + 1) * 512], in_=ps[:],
                        func=AF.Relu, bias=b1_sb[:, ht:ht + 1],
                    )

            # h [n_part, 8(nt), C] = t1 @ w2 + b2
            h_sb = per.tile([128, NT, C], F32, tag="h_sb")
            for nt in range(NT):
                ps = psum_sm.tile([128, C], F32, tag="ps_sm")
                for ht in range(HID // 128):
                    nc.tensor.matmul(
                        ps[:], lhsT=t1T[:, ht, nt * 128:(nt + 1) * 128],
                        rhs=w2_sb[:, ht, :],
                        start=(ht == 0), stop=(ht == HID // 128 - 1),
                    )
                nc.vector.tensor_add(out=h_sb[:, nt, :], in0=ps[:], in1=b2_bc[:])

            # ---------------- new_e (jt-sharded) + AllGather ----------------
            eigw_bc = bcast("eig_w", KPOW * (DIM + 1), "eigw_bc")
            eigb_bc = bcast("eig_b", KPOW, "eigb_bc")
            alpha_bc = bcast("alpha_w", KPOW, "alpha_bc")

            w2s = per.tile([128, KPOW, DIM // 2], F32, tag="w2s")
            w2c = per.tile([128, KPOW, DIM // 2], F32, tag="w2c")
            eigw_3d = eigw_bc[:].rearrange("p (k d) -> p k d", d=DIM + 1)
            alpha_b3 = alpha_bc[:].unsqueeze(2).broadcast_to([128, KPOW, DIM // 2])
            nc.vector.tensor_tensor(out=w2s[:], in0=alpha_b3, in1=eigw_3d[:, :, 1:1 + DIM // 2], op=ALU.mult)
            nc.vector.tensor_tensor(out=w2c[:], in0=alpha_b3, in1=eigw_3d[:, :, 1 + DIM // 2:DIM + 1], op=ALU.mult)
            w0t = per.tile([128, KPOW], F32, tag="w0t")
            nc.vector.tensor_tensor(out=w0t[:], in0=eigw_3d[:, :, 0], in1=eigb_bc[:], op=ALU.add)
            nc.vector.tensor_tensor(out=w0t[:], in0=w0t[:], in1=alpha_bc[:], op=ALU.mult)
            w0 = per.tile([128, 1], F32, tag="w0")
            nc.vector.tensor_reduce(out=w0[:], in_=w0t[:], axis=mybir.AxisListType.X, op=ALU.add)

            e_sb = per.tile([128, JTC], F32, tag="e_sb")
            nc.scalar.dma_start(out=e_sb[:], in_=io["e_js"].ap())
            pows = per.tile([128, JTC, KPOW], F32, tag="pows")
            nc.vector.tensor_copy(out=pows[:, :, 0], in_=e_sb[:])
            for k in range(1, KPOW):
                nc.vector.tensor_tensor(out=pows[:, :, k], in0=pows[:, :, k - 1], in1=e_sb[:], op=ALU.mult)

            WNE = JTC * KPOW * (DIM // 2)  # 1280
            pe_t = per.tile([128, JTC, KPOW, DIM // 2], F32, tag="pe_t")
            kq_t = per.tile([128, WNE], F32, tag="kq_t")
            trig = per.tile([128, WNE], F32, tag="trig")
            ne_s = per.tile([128, JTC], F32, tag="ne_s")
            ne_c = per.tile([128, JTC], F32, tag="ne_c")

            pows_b = pows[:].unsqueeze(3).broadcast_to([128, JTC, KPOW, DIM // 2])
            div_b = div_sb[:].unsqueeze(1).unsqueeze(1).broadcast_to([128, JTC, KPOW, DIM // 2])
            nc.vector.tensor_tensor(out=pe_t[:], in0=pows_b, in1=div_b, op=ALU.mult)
            pe_f = pe_t[:].rearrange("p a b c -> p (a b c)")
            nc.vector.tensor_scalar(out=kq_t[:], in0=pe_f, scalar1=INV_2PI, scalar2=MAGIC, op0=ALU.mult, op1=ALU.add)
            nc.vector.tensor_scalar_sub(out=kq_t[:], in0=kq_t[:], scalar1=MAGIC)
            # range-reduce pe in place: pe -= k*(c1+c2+c3)
            nc.vector.cody_waite_cascade(pe_f, pe_f, kq_t[:], CW_C1, CW_C2, CW_C3)

            w2s_b = w2s[:].rearrange("p k d -> p (k d)").unsqueeze(1).broadcast_to([128, JTC, KPOW * DIM // 2])
            w2c_b = w2c[:].rearrange("p k d -> p (k d)").unsqueeze(1).broadcast_to([128, JTC, KPOW * DIM // 2])
            pe_3 = pe_t[:].rearrange("p a b c -> p a (b c)")

            nc.scalar.activation(out=trig[:], in_=pe_f, func=AF.Sin)
            trig3 = trig[:].rearrange("p (a w) -> p a w", a=JTC)
            nc.vector.tensor_tensor(out=trig3, in0=trig3, in1=w2s_b, op=ALU.mult)
            nc.vector.tensor_reduce(out=ne_s[:], in_=trig3, axis=mybir.AxisListType.X, op=ALU.add)

            nc.vector.add_range_wrap(kq_t[:], pe_f, HALF_PI, PI_F, TWO_PI)
            nc.scalar.activation(out=trig[:], in_=kq_t[:], func=AF.Sin)
            nc.vector.tensor_tensor(out=trig3, in0=trig3, in1=w2c_b, op=ALU.mult)
            nc.vector.tensor_reduce(out=ne_c[:], in_=trig3, axis=mybir.AxisListType.X, op=ALU.add)

            nc.vector.tensor_tensor(out=ne_s[:], in0=ne_s[:], in1=ne_c[:], op=ALU.add)
            nc.vector.tensor_scalar_add(out=ne_s[:], in0=ne_s[:], scalar1=w0[:])

            ag_in = dram.tile([128 * JTC], F32, tag="ag_in")
            ag_out = dram.tile([N], F32, tag="ag_out", addr_space="Shared")
            new_e_sb = per.tile([128, JT], F32, tag="new_e_sb")

            def emit_ag():
                nc.sync.dma_start(out=ag_in[:].rearrange("(p w) -> p w", p=128), in_=ne_s[:])
                nc.gpsimd.collective_compute(
                    "AllGather", ALU.bypass, replica_groups=rg,
                    ins=[ag_in[:].opt()], outs=[ag_out[:].opt()],
                )
                nc.scalar.dma_start(
                    out=new_e_sb[:].rearrange("p (m w) -> p m w", w=JTC),
                    in_=ag_out[:].rearrange("(m p w) -> p m w", p=128, w=JTC),
                )

            # ---------------- LN1 + q/k/v + kTv partial (before pass 1 so kTv
            # rides the first AllReduce chunk) ----------------
            def layer_norm(src, dst, g_bc, b_bc):
                for nt in range(NT):
                    st = stats.tile([128, 6], F32, tag="ln_st")
                    nc.vector.bn_stats(out=st[:], in_=src[:, nt, :])
                    mv = stats.tile([128, 2], F32, tag="ln_mv")
                    nc.vector.bn_aggr(out=mv[:], in_=st[:])
                    rstd = stats.tile([128, 1], F32, tag="ln_rstd")
                    nc.scalar.activation(out=rstd[:], in_=mv[:, 1:2], func=AF.Sqrt, bias=eps_sb[:])
                    nc.vector.reciprocal(out=rstd[:], in_=rstd[:])
                    nc.vector.tensor_scalar(
                        out=dst[:, nt, :], in0=src[:, nt, :],
                        scalar1=mv[:, 0:1], scalar2=rstd[:],
                        op0=ALU.subtract, op1=ALU.mult,
                    )
                    nc.vector.tensor_tensor(out=dst[:, nt, :], in0=dst[:, nt, :], in1=g_bc[:], op=ALU.mult)
                    nc.vector.tensor_tensor(out=dst[:, nt, :], in0=dst[:, nt, :], in1=b_bc[:], op=ALU.add)

            mh_sb = per.tile([128, NT, C], F32, tag="mh_sb")
            layer_norm(h_sb, mh_sb, mhag_bc, mhab_bc)

            mhT = per.tile([128, ROWS], F32, tag="mhT")
            for nt in range(NT):
                tp = psum_tr.tile([128, 128], F32, tag="tr")
                nc.tensor.transpose(tp[:], mh_sb[:, nt, :], ident[:])
                nc.vector.tensor_copy(out=mhT[:, nt * 128:(nt + 1) * 128], in_=tp[:])

            qT = per.tile([128, ROWS], F32, tag="qT")
            for nch in range(ROWS // 512):
                ps = psum_sm.tile([128, 512], F32, tag="ps_sm")
                nc.tensor.matmul(ps[:], lhsT=wq_sb[:], rhs=mhT[:, nch * 512:(nch + 1) * 512], start=True, stop=True)
                nc.scalar.activation(out=qT[:, nch * 512:(nch + 1) * 512], in_=ps[:], func=AF.Identity, bias=bq_pp[:])

            k_sb = per.tile([128, NT, C], F32, tag="k_sb")
            v_sb = per.tile([128, NT, C], F32, tag="v_sb")
            for nt in range(NT):
                ps = psum_sm.tile([128, C], F32, tag="ps_sm")
                nc.tensor.matmul(ps[:], lhsT=mhT[:, nt * 128:(nt + 1) * 128], rhs=wk_sb[:], start=True, stop=True)
                nc.vector.tensor_add(out=k_sb[:, nt, :], in0=ps[:], in1=bk_bc[:])
                ps2 = psum_sm.tile([128, C], F32, tag="ps_sm")
                nc.tensor.matmul(ps2[:], lhsT=mhT[:, nt * 128:(nt + 1) * 128], rhs=wv_sb[:], start=True, stop=True)
                nc.vector.tensor_add(out=v_sb[:, nt, :], in0=ps2[:], in1=bv_bc[:])

            kTv_sb = per.tile([128, C], F32, tag="kTv_sb")
            pskv = psum_sm.tile([128, C], F32, tag="ps_sm")
            for nt in range(NT):
                nc.tensor.matmul(pskv[:], lhsT=k_sb[:, nt, :], rhs=v_sb[:, nt, :], start=(nt == 0), stop=(nt == NT - 1))
            nc.vector.tensor_copy(out=kTv_sb[:], in_=pskv[:])

            # ---------------- pass 1 + chunked AllReduce pipeline ----------------
            # utx^T columns complete panel-by-panel, so all-reduce them in NAR
            # chunks while later panels still stream; pass 2 consumes chunks as
            # they land. kTv rides chunk 0.
            h16_sb = per.tile([128, NT, C], BF16, tag="h16_sb")
            for nt in range(NT):
                nc.vector.tensor_copy(out=h16_sb[:, nt, :], in_=h_sb[:, nt, :])

            NAR = 4
            JPC = JP // NAR              # panels per chunk
            CW = N // NAR                # columns per chunk
            utxT = per.tile([128, N], BF16, tag="utxT")
            STASH_JT = JPC * (PW // 128)   # j-tiles covered by chunk 0
            u16stash = per.tile([128, STASH_JT, ROWS], BF16, tag="u16stash")
            u16 = dram.tile([ROWS, N], BF16, tag="u16")
            u_r = io["u_s"].ap().rearrange("(t p) j -> p t j", p=128)
            u16_r = u16[:].rearrange("(t p) j -> p t j", p=128)

            ar_ins, ar_outs = [], []
            for c in range(NAR):
                ari = dram.tile([128, CW], BF16, tag=f"ar_in{c}", name=f"ar_in{c}")
                aro = dram.tile([128, CW], BF16, tag=f"ar_out{c}", name=f"ar_out{c}",
                                addr_space="Shared")
                ar_ins.append(ari)
                ar_outs.append(aro)
            ktv_in = dram.tile([128, C], F32, tag="ktv_in")
            ktv_out = dram.tile([128, C], F32, tag="ktv_out", addr_space="Shared")

            def emit_chunk_ar(c):
                # input copy on sync; trigger on gpsimd (required engine for
                # collectives); result load-back on scalar HWDGE so the sync
                # FIFO (panel writes) and Pool FIFO (panel loads) don't stall
                # on collective completion.
                nc.sync.dma_start(out=ar_ins[c][:], in_=utxT[:, c * CW:(c + 1) * CW])
                nc.gpsimd.collective_compute(
                    "AllReduce", ALU.add, replica_groups=rg,
                    ins=[ar_ins[c][:].opt()], outs=[ar_outs[c][:].opt()],
                )
                nc.scalar.dma_start(out=utxT[:, c * CW:(c + 1) * CW], in_=ar_outs[c][:])

            def emit_ktv_ar():
                nc.sync.dma_start(out=ktv_in[:], in_=kTv_sb[:])
                nc.gpsimd.collective_compute(
                    "AllReduce", ALU.add, replica_groups=rg,
                    ins=[ktv_in[:].opt()], outs=[ktv_out[:].opt()],
                )
                nc.scalar.dma_start(out=kTv_sb[:], in_=ktv_out[:])

            for jp in range(JP):
                panel = pan.tile([128, NT, PW], BF16, tag="panel")
                nc.gpsimd.dma_start(out=panel[:], in_=u_r[:, :, jp * PW:(jp + 1) * PW])
                # stagger collective triggers two panels into the next chunk so
                # the gpsimd sequencer's wait overlaps in-flight panel loads
                if jp == 2:
                    emit_ag()
                if jp >= JPC + 1 and (jp - JPC - 1) % JPC == 0 and (jp - JPC - 1) // JPC < NAR - 1:
                    c = (jp - JPC - 1) // JPC
                    emit_chunk_ar(c)
                    if c == 0:
                        emit_ktv_ar()
                ps = psum_sm.tile([128, PW], F32, tag="ps_sm")
                for nt in range(NT):
                    nc.tensor.matmul(
                        ps[:], lhsT=h16_sb[:, nt, :],
                        rhs=panel[:, nt, :],
                        start=(nt == 0), stop=(nt == NT - 1),
                    )
                nc.vector.tensor_copy(out=utxT[:, jp * PW:(jp + 1) * PW], in_=ps[:])
                if jp < JPC:
                    # chunk 0: transpose panels on-chip into the SBUF stash
                    # instead of round-tripping them through DRAM
                    for jtl in range(PW // 128):
                        jt = jp * (PW // 128) + jtl
                        for nt in range(NT):
                            tps = psum_tr.tile([128, 128], BF16, tag="tr", name="tps")
                            nc.tensor.transpose(
                                tps[:], panel[:, nt, jtl * 128:(jtl + 1) * 128], ident16[:])
                            nc.vector.tensor_copy(
                                out=u16stash[:, jt, nt * 128:(nt + 1) * 128], in_=tps[:])
                else:
                    nc.sync.dma_start(out=u16_r[:, :, jp * PW:(jp + 1) * PW], in_=panel[:])
            emit_chunk_ar(NAR - 1)

            # ---------------- pass 2: h_fur^T = sum_jt g16[jt]^T @ u16T[jt] ----------------
            JTCW = CW // 128             # jt tiles per chunk
            hfur_ps = psum_acc.tile([128, ROWS], F32, tag="hfur")
            u16_2d = u16[:]
            for c in range(NAR):
                for jtl in range(JTCW):
                    jt = c * JTCW + jtl
                    tp = psum_tr.tile([128, 128], BF16, tag="tr", name="tp16")
                    nc.tensor.transpose(tp[:], utxT[:, jt * 128:(jt + 1) * 128], ident16[:])
                    g16 = g16_pool.tile([128, 128], BF16, tag="g16")
                    nc.vector.tensor_scalar_mul(out=g16[:], in0=tp[:], scalar1=new_e_sb[:, jt:jt + 1])
                    if jt < STASH_JT:
                        ut = u16stash[:, jt, :]
                    else:
                        ut = u16t_pool.tile([128, ROWS], BF16, tag="ut")
                        nc.scalar.dma_start(out=ut[:], in_=u16_2d[:, jt * 128:(jt + 1) * 128], transpose=True)
                    for hf in range(ROWS // 512):
                        nc.tensor.matmul(
                            hfur_ps[:, hf * 512:(hf + 1) * 512], lhsT=g16[:],
                            rhs=ut[:, hf * 512:(hf + 1) * 512],
                            start=(jt == 0), stop=(jt == JT - 1),
                            skip_group_check=True,
                        )

            # ---------------- att^T, att2^T + h_fur^T -> s^T; h1 = h + s ----------------
            hfurT = per.tile([128, ROWS], F32, tag="hfurT")
            nc.vector.tensor_copy(out=hfurT[:], in_=hfur_ps[:])

            attT = per.tile([128, ROWS], F32, tag="attT")
            for nch in range(ROWS // 512):
                ps = psum_sm.tile([128, 512], F32, tag="ps_sm")
                nc.tensor.matmul(ps[:], lhsT=kTv_sb[:], rhs=qT[:, nch * 512:(nch + 1) * 512], start=True, stop=True)
                nc.vector.tensor_copy(out=attT[:, nch * 512:(nch + 1) * 512], in_=ps[:])

            sT = per.tile([128, ROWS], F32, tag="sT")
            for nch in range(ROWS // 512):
                ps = psum_sm.tile([128, 512], F32, tag="ps_sm")
                nc.tensor.matmul(ps[:], lhsT=wo_sb[:], rhs=attT[:, nch * 512:(nch + 1) * 512], start=True, stop=True)
                nc.vector.scalar_tensor_tensor(
                    out=sT[:, nch * 512:(nch + 1) * 512], in0=ps[:], scalar=bo_pp[:],
                    in1=hfurT[:, nch * 512:(nch + 1) * 512],
                    op0=ALU.add, op1=ALU.add,
                )

            h1_sb = per.tile([128, NT, C], F32, tag="h1_sb")
            for nt in range(NT):
                tp = psum_tr.tile([128, 128], F32, tag="tr")
                nc.tensor.transpose(tp[:], sT[:, nt * 128:(nt + 1) * 128], ident[:])
                nc.vector.tensor_add(out=h1_sb[:, nt, :], in0=tp[:], in1=h_sb[:, nt, :])

            # ---------------- FFN: h_out = h1 + (gelu(LN(h1)@f1+b1))@f2+b2 ----------------
            mh2_sb = per.tile([128, NT, C], F32, tag="mh2_sb")
            layer_norm(h1_sb, mh2_sb, ffng_bc, ffnb_bc)
            mh2T = per.tile([128, ROWS], F32, tag="mh2T")
            for nt in range(NT):
                tp = psum_tr.tile([128, 128], F32, tag="tr")
                nc.tensor.transpose(tp[:], mh2_sb[:, nt, :], ident[:])
                nc.vector.tensor_copy(out=mh2T[:, nt * 128:(nt + 1) * 128], in_=tp[:])

            gzT = per.tile([128, ROWS], F32, tag="gzT")
            for nch in range(ROWS // 512):
                ps = psum_sm.tile([128, 512], F32, tag="ps_sm")
                nc.tensor.matmul(ps[:], lhsT=f1w_sb[:], rhs=mh2T[:, nch * 512:(nch + 1) * 512], start=True, stop=True)
                nc.scalar.activation(out=gzT[:, nch * 512:(nch + 1) * 512], in_=ps[:], func=AF.Gelu, bias=f1b_pp[:])

            f2T = per.tile([128, ROWS], F32, tag="f2T")
            for nch in range(ROWS // 512):
                ps = psum_sm.tile([128, 512], F32, tag="ps_sm")
                nc.tensor.matmul(ps[:], lhsT=f2w_sb[:], rhs=gzT[:, nch * 512:(nch + 1) * 512], start=True, stop=True)
                nc.scalar.activation(out=f2T[:, nch * 512:(nch + 1) * 512], in_=ps[:], func=AF.Identity, bias=f2b_pp[:])

            hout_sb = per.tile([128, NT, C], F32, tag="hout_sb")
            for nt in range(NT):
                tp = psum_tr.tile([128, 128], F32, tag="tr")
                nc.tensor.transpose(tp[:], f2T[:, nt * 128:(nt + 1) * 128], ident[:])
                nc.vector.tensor_add(out=hout_sb[:, nt, :], in0=tp[:], in1=h1_sb[:, nt, :])

            nc.sync.dma_start(out=y.ap().rearrange("(t p) c -> p t c", p=128), in_=hout_sb[:])

    nc.compile()
    return nc


_NC = None


def _get_nc():
    global _NC
    if _NC is None:
        _NC = _build(bacc.Bacc("TRN2", target_bir_lowering=False, debug=False, num_devices=NCORES))
    return _NC


def make_in_maps(inputs):
    e = np.ascontiguousarray(np.asarray(inputs["e"], dtype=np.float32))
    u = np.asarray(inputs["u"], dtype=np.float32)
    x = np.asarray(inputs["x"], dtype=np.float32)
    e_resh = np.ascontiguousarray(e.reshape(JT, 128).T)  # [p, jt] = e[jt*128+p]

    weights = {
        name: np.ascontiguousarray(np.asarray(inputs[name], dtype=np.float32))
        for name, _ in WEIGHT_NAMES
    }

    in_maps = []
    for m in range(NCORES):
        im = {
            "u_s": np.ascontiguousarray(u[m * ROWS:(m + 1) * ROWS]),
            "x_s": np.ascontiguousarray(x[m * ROWS:(m + 1) * ROWS]),
            "e_js": np.ascontiguousarray(e_resh[:, m * JTC:(m + 1) * JTC]),
        }
        im.update(weights)
        in_maps.append(im)
    return in_maps


def kernel(**inputs):
    nc = _get_nc()
    in_maps = make_in_maps(inputs)

    import os
    trace = bool(int(os.environ.get("KERNEL_TRACE", "0")))
    res = run_bass_kernel_spmd(nc, in_maps, core_ids=list(range(NCORES)), trace=trace)
    if trace and res.exec_time_ns is not None:
        print(f"HW exec time: {res.exec_time_ns} ns")
        if res.instructions_and_trace is not None:
            print("trace:", res.instructions_and_trace[1])
    out = np.concatenate([r["y"] for r in res.results], axis=0)
    return out.astype(np.float32)
